# revision 19
# baseline (speedup 1.0000x reference)
"""Trainium2 Bass kernel for the CrossAttention reference module.

  claim = x[claim_index]; evidence = x[evidence_index]
  wc = claim @ Wc + bc; we = evidence @ We + be
  S = wc @ we.T + blockdiag_mask(batch[claim_index], batch[evidence_index])
  A = softmax(S, -1); cn = A @ evidence
  a = concat([claim, cn, claim-cn, claim*cn]) @ Wa + ba
  out = segment_mean(a, batch[claim_index], 64)

Sharding: claim rows (4096) are split 512 per NeuronCore across 8 cores;
each core gets the evidence rows of its graphs.  Each core computes a
partial segment sum [64, 512]; the host sums the 8 partials, divides by
the per-graph claim counts, and adds ba.

Host staging: claims/evidence are gathered on the host and shipped
pre-transposed in bf16, in the exact SBUF layouts the matmuls want, so
the device does no gathers, transposes, or dtype converts.

Masking trick: the 64-dim projections are augmented with 64 extra one-hot
"graph id" dims scaled by 32.0, so the score matmul produces
S + 1024*same_graph.  exp(S_aug - 1074) underflows to exactly 0 for
cross-graph pairs and equals exp(S - 50) for same-graph pairs -- a
row-constant shift softmax ignores.

Windowing: claims and evidence are sorted by graph, so claim tile t
(128 claims) only attends to a contiguous window of evidence subtiles.
Windows are computed from the data (union over cores -> one program for
all cores); out-of-graph rows inside a window are killed by the mask.

Softmax normalization is folded to the end: with unnormalized n = P@ev
and r = 1/rowsum,
  a = concat(c, n*r, c-n*r, c*(n*r)) @ Wa
    = c @ (Wa0+Wa2) + r * (n @ (Wa1-Wa2) + (c*n) @ Wa3)
so the [Nc, Ne] attention matrix is never rescaled, and the a-matmul
contraction shrinks from 2048 to 1536.
"""

import sys

if "/opt/trn_rl_repo" not in sys.path:
    sys.path.insert(0, "/opt/trn_rl_repo")

import ml_dtypes
import numpy as np

import concourse.bass as bass
import concourse.mybir as mybir
import concourse.tile as tile
from concourse.bass_utils import run_bass_kernel_spmd
from concourse.vector_clock import ScopedClock

P = 128
NHID = 512
PROJ = 64
NC_ALL = 4096
NG = 64
N_CORES = 8
NC_LOC = NC_ALL // N_CORES  # 512 claims per core
KT_H = NHID // P            # 4 hidden k-tiles
CT = NC_LOC // P            # 4 claim tiles per core
MAG = 32.0                  # sqrt(1024): one-hot scale
EXP_BIAS = -(MAG * MAG + 50.0)  # exp(S + 1024 - 1074) = exp(S - 50)

f32 = mybir.dt.float32
bf16 = mybir.dt.bfloat16
i32 = mybir.dt.int32
AF = mybir.ActivationFunctionType
ALU = mybir.AluOpType
BF = ml_dtypes.bfloat16


class _PatchedTileContext(tile.TileContext):
    """Workaround: this neuronxcc/walrus build rejects InstDrain carrying
    sync waits ("Too many sync wait commands").  Collect the final drain's
    waits on nops (one wait each) and emit the drain itself wait-free."""

    def _drain_and_barrier(self, tick_clock, wait_clock):
        nc = self.nc
        nop0 = nc.sync.nop(nofuse=True)
        wait_clock.add_sem_waits(nop0.ins, ScopedClock({None: tick_clock.global_clock}))
        si = nop0.ins.sync_info
        waits = list(si.on_wait) if si and si.on_wait else []
        if si and len(waits) > 1:
            del si.on_wait[1:]
            for w in waits[1:]:
                extra = nc.sync.nop(nofuse=True)
                if extra.ins.sync_info is None:
                    extra.ins.sync_info = mybir.SyncInfo(on_wait=[w], on_update=[])
                else:
                    extra.ins.sync_info.on_wait.append(w)
        drain_inst = nc.sync.drain()
        wait_clock.add_sem_waits(
            drain_inst.ins, ScopedClock({None: tick_clock.global_clock})
        )
        dsi = drain_inst.ins.sync_info
        if dsi and dsi.on_wait:
            del dsi.on_wait[:]
        nc.all_engine_barrier()
        popped = nc._tile_sem_poison_stack.pop()
        assert popped is self._sem_poison
        nc.clear_and_free_semaphores(list(self.sems.allocated().values()))
        nc.all_engine_barrier()


def _split_excess_waits(nc: bass.Bass, limit: int = 1) -> None:
    """This walrus build rejects instructions carrying more than ~1 sync
    wait.  Move excess waits onto injected same-engine nops (engines are
    in-order, so gating a preceding nop gates the instruction)."""
    for f in nc.m.functions:
        for bb in f.blocks:
            new_insts = []
            for inst in bb.instructions:
                si = getattr(inst, "sync_info", None)
                if si is not None and si.on_wait and len(si.on_wait) > limit:
                    keep = list(si.on_wait[-limit:])
                    excess = list(si.on_wait[:-limit])
                    for w in excess:
                        nop = mybir.InstNoOp(
                            name=f"I-{nc.next_id()}", engine=inst.engine,
                            ins=[], outs=[],
                            sync_info=mybir.SyncInfo(on_wait=[w], on_update=[]))
                        new_insts.append(nop)
                    del si.on_wait[:]
                    si.on_wait.extend(keep)
                new_insts.append(inst)
            bb.instructions[:] = new_insts


def build_nc(reps: int = 1, ne_loc: int = 1280,
             windows: tuple = ((0, 4), (1, 5), (2, 6), (5, 5))) -> bass.Bass:
    ET = ne_loc // P
    offs = [0]
    for _, n in windows:
        offs.append(offs[-1] + n)
    TOT = offs[-1]          # total score subtiles across the 4 claim tiles

    nc = bass.Bass("TRN2", target_bir_lowering=False, debug=False,
                   num_devices=N_CORES)

    evn_d = nc.dram_tensor("evn", [P, ET * NHID], bf16, kind="ExternalInput").ap()
    evt_d = nc.dram_tensor("evt", [P, KT_H * ne_loc], bf16, kind="ExternalInput").ap()
    clt_d = nc.dram_tensor("clt", [P, CT * KT_H * P], bf16, kind="ExternalInput").ap()
    clk_d = nc.dram_tensor("clk", [P, KT_H * NC_LOC], bf16, kind="ExternalInput").ap()
    w0_d = nc.dram_tensor("w0", [P, KT_H * NHID], bf16, kind="ExternalInput").ap()
    w1_d = nc.dram_tensor("w1", [P, KT_H * NHID], bf16, kind="ExternalInput").ap()
    w3_d = nc.dram_tensor("w3", [P, KT_H * NHID], bf16, kind="ExternalInput").ap()
    wcb_d = nc.dram_tensor("wcb", [P, KT_H * PROJ], bf16, kind="ExternalInput").ap()
    web_d = nc.dram_tensor("web", [P, KT_H * PROJ], bf16, kind="ExternalInput").ap()
    bc_d = nc.dram_tensor("bc", [PROJ, 1], f32, kind="ExternalInput").ap()
    be_d = nc.dram_tensor("be", [PROJ, 1], f32, kind="ExternalInput").ap()
    ebr_d = nc.dram_tensor("ebr", [1, ne_loc], bf16, kind="ExternalInput").ap()
    cbr_d = nc.dram_tensor("cbr", [1, NC_LOC], bf16, kind="ExternalInput").ap()
    cbc_d = nc.dram_tensor("cbc", [P, CT], f32, kind="ExternalInput").ap()
    seg_d = nc.dram_tensor("seg", [NG, NHID], f32, kind="ExternalOutput").ap()

    with _PatchedTileContext(nc) as tc:
        with (
            tc.tile_pool(name="const", bufs=1) as cpool,
            tc.tile_pool(name="big", bufs=1) as bigpool,
            tc.tile_pool(name="psS", bufs=2, space="PSUM") as psS,
            tc.tile_pool(name="psR", bufs=2, space="PSUM") as psR,
            tc.tile_pool(name="psV", bufs=2, space="PSUM") as psV,
            tc.tile_pool(name="psA", bufs=2, space="PSUM") as psA,
        ):
            # ---------- persistent constants ----------
            ones_row = cpool.tile([1, PROJ], bf16)
            nc.gpsimd.memset(ones_row[:], 1.0)
            ones_col = cpool.tile([P, 1], bf16)
            nc.gpsimd.memset(ones_col[:], 1.0)
            exp_bias = cpool.tile([P, 1], f32)
            nc.gpsimd.memset(exp_bias[:], EXP_BIAS)
            g_col = cpool.tile([PROJ, 1], f32)
            iota_row = cpool.tile([P, NG], f32)
            bc_sb = cpool.tile([PROJ, 1], f32)
            nc.sync.dma_start(bc_sb[:], bc_d[:])
            be_sb = cpool.tile([PROJ, 1], f32)
            nc.sync.dma_start(be_sb[:], be_d[:])
            wc_b = cpool.tile([P, KT_H, PROJ], bf16)
            nc.sync.dma_start(wc_b[:], wcb_d.rearrange("p (k m) -> p k m", k=KT_H))
            we_b = cpool.tile([P, KT_H, PROJ], bf16)
            nc.sync.dma_start(we_b[:], web_d.rearrange("p (k m) -> p k m", k=KT_H))
            cbc = cpool.tile([P, CT], f32)
            nc.sync.dma_start(cbc[:], cbc_d[:])

            # ---------- persistent big buffers ----------
            evn = bigpool.tile([P, ET, NHID], bf16)
            nc.sync.dma_start(evn[:], evn_d.rearrange("p (e h) -> p e h", e=ET))
            evt = bigpool.tile([P, KT_H, ne_loc], bf16)
            nc.sync.dma_start(evt[:], evt_d.rearrange("p (k e) -> p k e", k=KT_H))
            clt = bigpool.tile([P, CT, KT_H * P], bf16)
            nc.sync.dma_start(clt[:], clt_d.rearrange("p (t j) -> p t j", t=CT))
            clk = bigpool.tile([P, KT_H, NC_LOC], bf16)
            nc.sync.dma_start(clk[:], clk_d.rearrange("p (k j) -> p k j", k=KT_H))
            w0 = bigpool.tile([P, KT_H, NHID], bf16)
            nc.sync.dma_start(w0[:], w0_d.rearrange("p (k n) -> p k n", k=KT_H))
            w1 = bigpool.tile([P, KT_H, NHID], bf16)
            nc.sync.dma_start(w1[:], w1_d.rearrange("p (k n) -> p k n", k=KT_H))
            w3 = bigpool.tile([P, KT_H, NHID], bf16)
            nc.sync.dma_start(w3[:], w3_d.rearrange("p (k n) -> p k n", k=KT_H))

            we_aug = bigpool.tile([P, ne_loc], bf16)   # [64 proj | 64 onehot]
            wc_aug = bigpool.tile([P, NC_LOC], bf16)
            p_sb = bigpool.tile([P, TOT * P], bf16)    # exp'd scores, transposed
            cnt = bigpool.tile([P, CT, KT_H * P], bf16)  # unnormalized cn^T
            ctn = bigpool.tile([P, CT, KT_H * P], bf16)  # clT * cnT
            r_sb = bigpool.tile([P, CT], f32)          # 1/rowsum per claim
            a_sb = bigpool.tile([P, CT, NHID], bf16)
            oh_seg = bigpool.tile([P, CT, NG], bf16)

            # ---------- prologue ----------
            with tc.tile_pool(name="pro", bufs=1) as pr:
                g_col_i = pr.tile([PROJ, 1], i32)
                nc.gpsimd.iota(g_col_i[:], pattern=[[0, 1]], base=0,
                               channel_multiplier=1)
                nc.vector.tensor_copy(g_col[:], g_col_i[:])
                iota_row_i = pr.tile([P, NG], i32)
                nc.gpsimd.iota(iota_row_i[:], pattern=[[1, NG]], base=0,
                               channel_multiplier=0)
                nc.vector.tensor_copy(iota_row[:], iota_row_i[:])

                ebr = pr.tile([1, ne_loc], bf16)
                nc.sync.dma_start(ebr[:], ebr_d[:])
                cbr = pr.tile([1, NC_LOC], bf16)
                nc.sync.dma_start(cbr[:], cbr_d[:])

                # one-hot graph rows of the augmented projections
                for c0 in range(0, ne_loc, NHID):
                    cw = min(NHID, ne_loc - c0)
                    b_ps = psS.tile([PROJ, NHID], f32, tag="s")
                    nc.tensor.matmul(b_ps[:, :cw], ones_row[:],
                                     ebr[:, c0:c0 + cw], start=True, stop=True)
                    nc.vector.tensor_scalar(
                        out=we_aug[PROJ:, c0:c0 + cw], in0=b_ps[:, :cw],
                        scalar1=g_col[:], scalar2=MAG, op0=ALU.is_equal,
                        op1=ALU.mult)
                b_ps = psS.tile([PROJ, NHID], f32, tag="s")
                nc.tensor.matmul(b_ps[:, :NC_LOC], ones_row[:], cbr[:],
                                 start=True, stop=True)
                nc.vector.tensor_scalar(
                    out=wc_aug[PROJ:, :], in0=b_ps[:, :NC_LOC], scalar1=g_col[:],
                    scalar2=MAG, op0=ALU.is_equal, op1=ALU.mult)
                for t in range(CT):
                    nc.vector.tensor_scalar(
                        out=oh_seg[:, t, :], in0=iota_row[:],
                        scalar1=cbc[:, t:t + 1], scalar2=None, op0=ALU.is_equal)

            # ---------- main body ----------
            with tc.tile_pool(name="work", bufs=2) as wpool:
                def proj_chunk(c0):
                    cw = min(NHID, ne_loc - c0)
                    ps = psA.tile([PROJ, NHID], f32, tag="A")
                    for k in range(KT_H):
                        nc.tensor.matmul(ps[:, :cw], we_b[:, k, :],
                                         evt[:, k, c0:c0 + cw],
                                         start=(k == 0), stop=(k == KT_H - 1))
                    nc.scalar.activation(we_aug[:PROJ, c0:c0 + cw],
                                         ps[:, :cw], AF.Identity, bias=be_sb[:])

                def body():
                    # evidence projection chunk 0, then claims, then the rest:
                    # scores for tile 0 only need the first window's subtiles.
                    proj_chunk(0)
                    ps = psA.tile([PROJ, NHID], f32, tag="A")
                    for k in range(KT_H):
                        nc.tensor.matmul(ps[:, :NC_LOC], wc_b[:, k, :], clk[:, k, :],
                                         start=(k == 0), stop=(k == KT_H - 1))
                    nc.scalar.activation(wc_aug[:PROJ, :], ps[:, :NC_LOC],
                                         AF.Identity, bias=bc_sb[:])
                    for c0 in range(NHID, ne_loc, NHID):
                        proj_chunk(c0)

                    # scores^T + exp, per claim tile, windowed
                    for t in range(CT):
                        lo, nsub = windows[t]
                        for c0 in range(0, nsub, 4):
                            cw = min(4, nsub - c0)
                            sp = psS.tile([P, 4 * P], f32, tag="s")
                            for j in range(cw):
                                s = lo + c0 + j
                                nc.tensor.matmul(
                                    sp[:, j * P:(j + 1) * P],
                                    we_aug[:, s * P:(s + 1) * P],
                                    wc_aug[:, t * P:(t + 1) * P],
                                    start=True, stop=True)
                            nc.scalar.activation(
                                p_sb[:, (offs[t] + c0) * P:(offs[t] + c0 + cw) * P],
                                sp[:, :cw * P], AF.Exp, bias=exp_bias[:])

                    # back half, pipelined per claim tile:
                    # rowsum -> PV -> cn copies -> c*n -> a-matmuls -> combine
                    for t in range(CT):
                        lo, nsub = windows[t]
                        rs = psR.tile([P, 1], f32, tag="rs")
                        for j in range(nsub):
                            nc.tensor.matmul(
                                rs[:], p_sb[:, (offs[t] + j) * P:(offs[t] + j + 1) * P],
                                ones_col[:], start=(j == 0), stop=(j == nsub - 1))
                        nc.vector.reciprocal(r_sb[:, t:t + 1], rs[:])

                        for h in range(KT_H):
                            pv = psV.tile([P, P], f32, tag="pv")
                            for j in range(nsub):
                                s = lo + j
                                nc.tensor.matmul(
                                    pv[:],
                                    evn[:, s, h * P:(h + 1) * P],
                                    p_sb[:, (offs[t] + j) * P:(offs[t] + j + 1) * P],
                                    start=(j == 0), stop=(j == nsub - 1))
                            if h % 2 == 0:
                                nc.scalar.copy(cnt[:, t, h * P:(h + 1) * P], pv[:])
                            else:
                                nc.vector.tensor_copy(cnt[:, t, h * P:(h + 1) * P],
                                                      pv[:])
                        nc.vector.tensor_tensor(out=ctn[:, t, :], in0=clt[:, t, :],
                                                in1=cnt[:, t, :], op=ALU.mult)

                        A0 = psA.tile([P, NHID], f32, tag="A")
                        for k in range(KT_H):
                            nc.tensor.matmul(A0[:], clt[:, t, k * P:(k + 1) * P],
                                             w0[:, k, :], start=(k == 0),
                                             stop=(k == KT_H - 1))
                        A1 = psS.tile([P, 4 * P], f32, tag="s")
                        for k in range(KT_H):
                            nc.tensor.matmul(A1[:], cnt[:, t, k * P:(k + 1) * P],
                                             w1[:, k, :], start=(k == 0), stop=False)
                        for k in range(KT_H):
                            nc.tensor.matmul(A1[:], ctn[:, t, k * P:(k + 1) * P],
                                             w3[:, k, :], start=False,
                                             stop=(k == KT_H - 1))
                        a_tmp = wpool.tile([P, NHID], bf16, tag="atmp")
                        nc.scalar.activation(a_tmp[:], A1[:], AF.Identity,
                                             scale=r_sb[:, t:t + 1])
                        nc.vector.tensor_tensor(out=a_sb[:, t, :], in0=a_tmp[:],
                                                in1=A0[:], op=ALU.add)

                    # segment sum via one-hot matmul
                    sg = psV.tile([NG, NHID], f32, tag="pv")
                    for t in range(CT):
                        nc.tensor.matmul(sg[:], oh_seg[:, t, :], a_sb[:, t, :],
                                         start=(t == 0), stop=(t == CT - 1))
                    seg_sb = wpool.tile([NG, NHID], f32, tag="seg")
                    nc.scalar.copy(seg_sb[:], sg[:])
                    nc.sync.dma_start(seg_d[:], seg_sb[:])

                if reps == 1:
                    body()
                else:
                    unroll = 4 if reps % 4 == 0 else 1
                    with tc.For_i(0, reps // unroll):
                        for _ in range(unroll):
                            body()
    _split_excess_waits(nc)
    return nc


def make_in_maps(inputs: dict):
    """Host-side gather/transpose/convert + per-core input maps."""
    batch = np.asarray(inputs["batch"]).astype(np.int64)
    ci = np.asarray(inputs["claim_index"]).astype(np.int64)
    ei = np.asarray(inputs["evidence_index"]).astype(np.int64)
    x = np.asarray(inputs["x"], dtype=np.float32)
    cb = batch[ci]
    eb = batch[ei]
    counts = np.bincount(cb, minlength=NG).astype(np.float32)
    ba = np.asarray(inputs["ba"], dtype=np.float32).reshape(NHID)

    order = np.argsort(cb, kind="stable")
    ci, cb = ci[order], cb[order]
    eorder = np.argsort(eb, kind="stable")
    ei, eb = ei[eorder], eb[eorder]

    Wc = np.asarray(inputs["Wc"], np.float32)
    We = np.asarray(inputs["We"], np.float32)
    Wa = np.asarray(inputs["Wa"], np.float32)
    W0 = (Wa[0:NHID] + Wa[2 * NHID:3 * NHID]).astype(BF)
    W1 = (Wa[NHID:2 * NHID] - Wa[2 * NHID:3 * NHID]).astype(BF)
    W3 = Wa[3 * NHID:].astype(BF)

    def ktile(w):  # [K, N] -> [128, KT*N] with K split over partitions
        k, n = w.shape
        return np.ascontiguousarray(
            w.reshape(k // P, P, n).transpose(1, 0, 2).reshape(P, -1))

    common = {
        "w0": ktile(W0), "w1": ktile(W1), "w3": ktile(W3),
        "wcb": ktile(Wc.astype(BF)), "web": ktile(We.astype(BF)),
        "bc": np.asarray(inputs["bc"], np.float32).reshape(PROJ, 1),
        "be": np.asarray(inputs["be"], np.float32).reshape(PROJ, 1),
    }

    # per-core evidence / claims + per-tile windows
    cores = []
    max_nev = 0
    for c in range(N_CORES):
        sl = slice(c * NC_LOC, (c + 1) * NC_LOC)
        cbs = cb[sl]
        sel = np.where(np.isin(eb, np.unique(cbs)))[0]
        cores.append((ci[sl], cbs, ei[sel], eb[sel]))
        max_nev = max(max_nev, len(sel))
    ne_loc = -(-max_nev // P) * P
    ET = ne_loc // P

    # union windows (in 128-subtiles) per claim tile
    windows = []
    for t in range(CT):
        lo_s, hi_s = ET, 0
        for (_, cbs, _, ebc) in cores:
            tg = cbs[t * P:(t + 1) * P]
            lo = np.searchsorted(ebc, tg[0], "left")
            hi = np.searchsorted(ebc, tg[-1], "right")
            lo_s = min(lo_s, int(lo) // P)
            hi_s = max(hi_s, -(-int(hi) // P))
        windows.append((lo_s, max(hi_s - lo_s, 1)))
    windows = tuple(windows)

    in_maps = []
    for c in range(N_CORES):
        cid, cbs, eidx, ebc = cores[c]
        nev = len(eidx)
        ev = np.zeros((ne_loc, NHID), np.float32)
        ev[:nev] = x[eidx]
        ebp = np.full(ne_loc, -2.0, np.float32)
        ebp[:nev] = ebc
        cl = x[cid]  # [512, 512]

        evb = ev.astype(BF)
        m = dict(common)
        m["evn"] = np.ascontiguousarray(
            evb.reshape(ET, P, NHID).transpose(1, 0, 2).reshape(P, -1))
        m["evt"] = np.ascontiguousarray(
            evb.T.reshape(KT_H, P, ne_loc).transpose(1, 0, 2).reshape(P, -1))
        clb = cl.astype(BF)
        # clt[p, t, h*128+j] = cl[t*128+j, h*128+p]
        m["clt"] = np.ascontiguousarray(
            clb.reshape(CT, P, KT_H, P).transpose(3, 0, 2, 1).reshape(P, -1))
        # clk[p, k, t*128+j] = cl[t*128+j, k*128+p]
        m["clk"] = np.ascontiguousarray(
            clb.T.reshape(KT_H, P, NC_LOC).transpose(1, 0, 2).reshape(P, -1))
        m["ebr"] = ebp.astype(BF).reshape(1, ne_loc)
        m["cbr"] = cbs.astype(np.float32).astype(BF).reshape(1, NC_LOC)
        m["cbc"] = np.ascontiguousarray(
            cbs.astype(np.float32).reshape(CT, P).T)
        in_maps.append(m)
    return in_maps, counts, ba, ne_loc, windows


def postprocess(results: list, counts: np.ndarray, ba: np.ndarray) -> np.ndarray:
    seg = np.zeros((NG, NHID), np.float64)
    for c in range(N_CORES):
        seg += results[c]["seg"].astype(np.float64)
    # segment_mean(a + ba) = segment_mean(a) + ba, except empty graphs stay 0
    out = seg / np.maximum(counts, 1.0)[:, None] + (counts > 0)[:, None] * ba[None, :]
    return out.astype(np.float32)


def kernel(**inputs) -> np.ndarray:
    in_maps, counts, ba, ne_loc, windows = make_in_maps(inputs)
    nc = build_nc(reps=1, ne_loc=ne_loc, windows=windows)
    res = run_bass_kernel_spmd(nc, in_maps, list(range(N_CORES)))
    return postprocess(res.results, counts, ba)


# revision 21
# speedup vs baseline: 1.1247x; 1.1247x over previous
"""Trainium2 Bass kernel for the CrossAttention reference module.

  claim = x[claim_index]; evidence = x[evidence_index]
  wc = claim @ Wc + bc; we = evidence @ We + be
  S = wc @ we.T + blockdiag_mask(batch[claim_index], batch[evidence_index])
  A = softmax(S, -1); cn = A @ evidence
  a = concat([claim, cn, claim-cn, claim*cn]) @ Wa + ba
  out = segment_mean(a, batch[claim_index], 64)

Sharding: claim rows (4096) are split 512 per NeuronCore across 8 cores;
each core gets the evidence rows of its graphs.  Each core computes a
partial segment sum [64, 512]; the host sums the 8 partials, divides by
the per-graph claim counts, and adds ba.

Host staging: claims/evidence are gathered on the host and shipped
pre-transposed in bf16, in the exact SBUF layouts the matmuls want, so
the device does no gathers, transposes, or dtype converts.

Masking trick: the 64-dim projections are augmented with 64 extra one-hot
"graph id" dims scaled by 32.0, so the score matmul produces
S + 1024*same_graph.  exp(S_aug - 1074) underflows to exactly 0 for
cross-graph pairs and equals exp(S - 50) for same-graph pairs -- a
row-constant shift softmax ignores.

Windowing: claims and evidence are sorted by graph, so claim tile t
(128 claims) only attends to a contiguous window of evidence subtiles.
Windows are computed from the data (union over cores -> one program for
all cores); out-of-graph rows inside a window are killed by the mask.

Softmax normalization is folded to the end: with unnormalized n = P@ev
and r = 1/rowsum,
  a = concat(c, n*r, c-n*r, c*(n*r)) @ Wa
    = c @ (Wa0+Wa2) + r * (n @ (Wa1-Wa2) + (c*n) @ Wa3)
so the [Nc, Ne] attention matrix is never rescaled, and the a-matmul
contraction shrinks from 2048 to 1536.
"""

import sys

if "/opt/trn_rl_repo" not in sys.path:
    sys.path.insert(0, "/opt/trn_rl_repo")

import ml_dtypes
import numpy as np

import concourse.bass as bass
import concourse.mybir as mybir
import concourse.tile as tile
from concourse.bass_utils import run_bass_kernel_spmd
from concourse.vector_clock import ScopedClock

P = 128
NHID = 512
PROJ = 64
NC_ALL = 4096
NG = 64
N_CORES = 8
NC_LOC = NC_ALL // N_CORES  # 512 claims per core
KT_H = NHID // P            # 4 hidden k-tiles
CT = NC_LOC // P            # 4 claim tiles per core
MAG = 32.0                  # sqrt(1024): one-hot scale
EXP_BIAS = -(MAG * MAG + 50.0)  # exp(S + 1024 - 1074) = exp(S - 50)

f32 = mybir.dt.float32
bf16 = mybir.dt.bfloat16
i32 = mybir.dt.int32
AF = mybir.ActivationFunctionType
ALU = mybir.AluOpType
BF = ml_dtypes.bfloat16


class _PatchedTileContext(tile.TileContext):
    """Workaround: this neuronxcc/walrus build rejects InstDrain carrying
    sync waits ("Too many sync wait commands").  Collect the final drain's
    waits on nops (one wait each) and emit the drain itself wait-free."""

    def _drain_and_barrier(self, tick_clock, wait_clock):
        nc = self.nc
        nop0 = nc.sync.nop(nofuse=True)
        wait_clock.add_sem_waits(nop0.ins, ScopedClock({None: tick_clock.global_clock}))
        si = nop0.ins.sync_info
        waits = list(si.on_wait) if si and si.on_wait else []
        if si and len(waits) > 1:
            del si.on_wait[1:]
            for w in waits[1:]:
                extra = nc.sync.nop(nofuse=True)
                if extra.ins.sync_info is None:
                    extra.ins.sync_info = mybir.SyncInfo(on_wait=[w], on_update=[])
                else:
                    extra.ins.sync_info.on_wait.append(w)
        drain_inst = nc.sync.drain()
        wait_clock.add_sem_waits(
            drain_inst.ins, ScopedClock({None: tick_clock.global_clock})
        )
        dsi = drain_inst.ins.sync_info
        if dsi and dsi.on_wait:
            del dsi.on_wait[:]
        nc.all_engine_barrier()
        popped = nc._tile_sem_poison_stack.pop()
        assert popped is self._sem_poison
        nc.clear_and_free_semaphores(list(self.sems.allocated().values()))
        nc.all_engine_barrier()


def _split_excess_waits(nc: bass.Bass, limit: int = 1) -> None:
    """This walrus build rejects instructions carrying more than ~1 sync
    wait.  Move excess waits onto injected same-engine nops (engines are
    in-order, so gating a preceding nop gates the instruction)."""
    for f in nc.m.functions:
        for bb in f.blocks:
            new_insts = []
            for inst in bb.instructions:
                si = getattr(inst, "sync_info", None)
                if si is not None and si.on_wait and len(si.on_wait) > limit:
                    keep = list(si.on_wait[-limit:])
                    excess = list(si.on_wait[:-limit])
                    for w in excess:
                        nop = mybir.InstNoOp(
                            name=f"I-{nc.next_id()}", engine=inst.engine,
                            ins=[], outs=[],
                            sync_info=mybir.SyncInfo(on_wait=[w], on_update=[]))
                        new_insts.append(nop)
                    del si.on_wait[:]
                    si.on_wait.extend(keep)
                new_insts.append(inst)
            bb.instructions[:] = new_insts


def build_nc(reps: int = 1, ne_loc: int = 1280,
             windows: tuple = ((0, 4), (1, 5), (2, 6), (5, 5))) -> bass.Bass:
    ET = ne_loc // P
    offs = [0]
    for _, n in windows:
        offs.append(offs[-1] + n)
    TOT = offs[-1]          # total score subtiles across the 4 claim tiles

    nc = bass.Bass("TRN2", target_bir_lowering=False, debug=False,
                   num_devices=N_CORES)

    evn_d = nc.dram_tensor("evn", [P, ET * NHID], bf16, kind="ExternalInput").ap()
    evt_d = nc.dram_tensor("evt", [P, KT_H * ne_loc], bf16, kind="ExternalInput").ap()
    clt_d = nc.dram_tensor("clt", [P, CT * KT_H * P], bf16, kind="ExternalInput").ap()
    clk_d = nc.dram_tensor("clk", [P, KT_H * NC_LOC], bf16, kind="ExternalInput").ap()
    w0_d = nc.dram_tensor("w0", [P, KT_H * NHID], bf16, kind="ExternalInput").ap()
    w1_d = nc.dram_tensor("w1", [P, KT_H * NHID], bf16, kind="ExternalInput").ap()
    w3_d = nc.dram_tensor("w3", [P, KT_H * NHID], bf16, kind="ExternalInput").ap()
    wcb_d = nc.dram_tensor("wcb", [P, KT_H * PROJ], bf16, kind="ExternalInput").ap()
    web_d = nc.dram_tensor("web", [P, KT_H * PROJ], bf16, kind="ExternalInput").ap()
    bc_d = nc.dram_tensor("bc", [PROJ, 1], f32, kind="ExternalInput").ap()
    be_d = nc.dram_tensor("be", [PROJ, 1], f32, kind="ExternalInput").ap()
    ebr_d = nc.dram_tensor("ebr", [1, ne_loc], bf16, kind="ExternalInput").ap()
    cbr_d = nc.dram_tensor("cbr", [1, NC_LOC], bf16, kind="ExternalInput").ap()
    cbc_d = nc.dram_tensor("cbc", [P, CT], f32, kind="ExternalInput").ap()
    seg_d = nc.dram_tensor("seg", [NG, NHID], f32, kind="ExternalOutput").ap()

    with _PatchedTileContext(nc) as tc:
        with (
            tc.tile_pool(name="const", bufs=1) as cpool,
            tc.tile_pool(name="big", bufs=1) as bigpool,
            tc.tile_pool(name="psS", bufs=2, space="PSUM") as psS,
            tc.tile_pool(name="psR", bufs=2, space="PSUM") as psR,
            tc.tile_pool(name="psV", bufs=2, space="PSUM") as psV,
            tc.tile_pool(name="psA", bufs=2, space="PSUM") as psA,
        ):
            # ---------- persistent constants ----------
            ones_row = cpool.tile([1, PROJ], bf16)
            nc.gpsimd.memset(ones_row[:], 1.0)
            ones_col = cpool.tile([P, 1], bf16)
            nc.gpsimd.memset(ones_col[:], 1.0)
            exp_bias = cpool.tile([P, 1], f32)
            nc.gpsimd.memset(exp_bias[:], EXP_BIAS)
            g_col = cpool.tile([PROJ, 1], f32)
            iota_row = cpool.tile([P, NG], f32)
            bc_sb = cpool.tile([PROJ, 1], f32)
            nc.sync.dma_start(bc_sb[:], bc_d[:])
            be_sb = cpool.tile([PROJ, 1], f32)
            nc.sync.dma_start(be_sb[:], be_d[:])
            wc_b = cpool.tile([P, KT_H, PROJ], bf16)
            nc.sync.dma_start(wc_b[:], wcb_d.rearrange("p (k m) -> p k m", k=KT_H))
            we_b = cpool.tile([P, KT_H, PROJ], bf16)
            nc.sync.dma_start(we_b[:], web_d.rearrange("p (k m) -> p k m", k=KT_H))
            cbc = cpool.tile([P, CT], f32)
            nc.sync.dma_start(cbc[:], cbc_d[:])

            # ---------- persistent big buffers ----------
            evn = bigpool.tile([P, ET, NHID], bf16)
            nc.sync.dma_start(evn[:], evn_d.rearrange("p (e h) -> p e h", e=ET))
            evt = bigpool.tile([P, KT_H, ne_loc], bf16)
            nc.sync.dma_start(evt[:], evt_d.rearrange("p (k e) -> p k e", k=KT_H))
            clt = bigpool.tile([P, CT, KT_H * P], bf16)
            nc.sync.dma_start(clt[:], clt_d.rearrange("p (t j) -> p t j", t=CT))
            clk = bigpool.tile([P, KT_H, NC_LOC], bf16)
            nc.sync.dma_start(clk[:], clk_d.rearrange("p (k j) -> p k j", k=KT_H))
            w0 = bigpool.tile([P, KT_H, NHID], bf16)
            nc.sync.dma_start(w0[:], w0_d.rearrange("p (k n) -> p k n", k=KT_H))
            w1 = bigpool.tile([P, KT_H, NHID], bf16)
            nc.sync.dma_start(w1[:], w1_d.rearrange("p (k n) -> p k n", k=KT_H))
            w3 = bigpool.tile([P, KT_H, NHID], bf16)
            nc.sync.dma_start(w3[:], w3_d.rearrange("p (k n) -> p k n", k=KT_H))

            we_aug = bigpool.tile([P, ne_loc], bf16)   # [64 proj | 64 onehot]
            wc_aug = bigpool.tile([P, NC_LOC], bf16)
            p_sb = bigpool.tile([P, TOT * P], bf16)    # exp'd scores, transposed
            cnt = bigpool.tile([P, CT, KT_H * P], bf16)  # unnormalized cn^T
            ctn = bigpool.tile([P, CT, KT_H * P], bf16)  # clT * cnT
            r_sb = bigpool.tile([P, CT], f32)          # 1/rowsum per claim
            a_sb = bigpool.tile([P, CT, NHID], bf16)
            oh_seg = bigpool.tile([P, CT, NG], bf16)

            # ---------- prologue ----------
            with tc.tile_pool(name="pro", bufs=1) as pr:
                g_col_i = pr.tile([PROJ, 1], i32)
                nc.gpsimd.iota(g_col_i[:], pattern=[[0, 1]], base=0,
                               channel_multiplier=1)
                nc.vector.tensor_copy(g_col[:], g_col_i[:])
                iota_row_i = pr.tile([P, NG], i32)
                nc.gpsimd.iota(iota_row_i[:], pattern=[[1, NG]], base=0,
                               channel_multiplier=0)
                nc.vector.tensor_copy(iota_row[:], iota_row_i[:])

                ebr = pr.tile([1, ne_loc], bf16)
                nc.sync.dma_start(ebr[:], ebr_d[:])
                cbr = pr.tile([1, NC_LOC], bf16)
                nc.sync.dma_start(cbr[:], cbr_d[:])

                # one-hot graph rows of the augmented projections
                for c0 in range(0, ne_loc, NHID):
                    cw = min(NHID, ne_loc - c0)
                    b_ps = psS.tile([PROJ, NHID], f32, tag="s")
                    nc.tensor.matmul(b_ps[:, :cw], ones_row[:],
                                     ebr[:, c0:c0 + cw], start=True, stop=True)
                    nc.vector.tensor_scalar(
                        out=we_aug[PROJ:, c0:c0 + cw], in0=b_ps[:, :cw],
                        scalar1=g_col[:], scalar2=MAG, op0=ALU.is_equal,
                        op1=ALU.mult)
                b_ps = psS.tile([PROJ, NHID], f32, tag="s")
                nc.tensor.matmul(b_ps[:, :NC_LOC], ones_row[:], cbr[:],
                                 start=True, stop=True)
                nc.vector.tensor_scalar(
                    out=wc_aug[PROJ:, :], in0=b_ps[:, :NC_LOC], scalar1=g_col[:],
                    scalar2=MAG, op0=ALU.is_equal, op1=ALU.mult)
                for t in range(CT):
                    nc.vector.tensor_scalar(
                        out=oh_seg[:, t, :], in0=iota_row[:],
                        scalar1=cbc[:, t:t + 1], scalar2=None, op0=ALU.is_equal)

            # ---------- main body ----------
            with tc.tile_pool(name="work", bufs=2) as wpool:
                def proj_chunk(c0):
                    cw = min(NHID, ne_loc - c0)
                    ps = psA.tile([PROJ, NHID], f32, tag="A")
                    for k in range(KT_H):
                        nc.tensor.matmul(ps[:, :cw], we_b[:, k, :],
                                         evt[:, k, c0:c0 + cw],
                                         start=(k == 0), stop=(k == KT_H - 1))
                    nc.scalar.activation(we_aug[:PROJ, c0:c0 + cw],
                                         ps[:, :cw], AF.Identity, bias=be_sb[:])

                def body():
                    # evidence projection chunk 0, then claims, then the rest:
                    # scores for tile 0 only need the first window's subtiles.
                    proj_chunk(0)
                    ps = psA.tile([PROJ, NHID], f32, tag="A")
                    for k in range(KT_H):
                        nc.tensor.matmul(ps[:, :NC_LOC], wc_b[:, k, :], clk[:, k, :],
                                         start=(k == 0), stop=(k == KT_H - 1))
                    nc.scalar.activation(wc_aug[:PROJ, :], ps[:, :NC_LOC],
                                         AF.Identity, bias=bc_sb[:])
                    for c0 in range(NHID, ne_loc, NHID):
                        proj_chunk(c0)

                    # scores^T + exp, per claim tile, windowed
                    for t in range(CT):
                        lo, nsub = windows[t]
                        for c0 in range(0, nsub, 4):
                            cw = min(4, nsub - c0)
                            sp = psS.tile([P, 4 * P], f32, tag="s")
                            for j in range(cw):
                                s = lo + c0 + j
                                nc.tensor.matmul(
                                    sp[:, j * P:(j + 1) * P],
                                    we_aug[:, s * P:(s + 1) * P],
                                    wc_aug[:, t * P:(t + 1) * P],
                                    start=True, stop=True)
                            nc.scalar.activation(
                                p_sb[:, (offs[t] + c0) * P:(offs[t] + c0 + cw) * P],
                                sp[:, :cw * P], AF.Exp, bias=exp_bias[:])

                    # back half, pipelined per claim tile:
                    # rowsum -> PV -> cn copies -> c*n -> a-matmuls -> combine
                    for t in range(CT):
                        lo, nsub = windows[t]
                        rs = psR.tile([P, 1], f32, tag="rs")
                        for j in range(nsub):
                            nc.tensor.matmul(
                                rs[:], p_sb[:, (offs[t] + j) * P:(offs[t] + j + 1) * P],
                                ones_col[:], start=(j == 0), stop=(j == nsub - 1))
                        nc.vector.reciprocal(r_sb[:, t:t + 1], rs[:])

                        for h in range(KT_H):
                            pv = psV.tile([P, P], f32, tag="pv")
                            for j in range(nsub):
                                s = lo + j
                                nc.tensor.matmul(
                                    pv[:],
                                    evn[:, s, h * P:(h + 1) * P],
                                    p_sb[:, (offs[t] + j) * P:(offs[t] + j + 1) * P],
                                    start=(j == 0), stop=(j == nsub - 1))
                            if h % 2 == 0:
                                nc.scalar.copy(cnt[:, t, h * P:(h + 1) * P], pv[:])
                            else:
                                nc.vector.tensor_copy(cnt[:, t, h * P:(h + 1) * P],
                                                      pv[:])
                        nc.vector.tensor_tensor(out=ctn[:, t, :], in0=clt[:, t, :],
                                                in1=cnt[:, t, :], op=ALU.mult)

                        A0 = psA.tile([P, NHID], f32, tag="A")
                        for k in range(KT_H):
                            nc.tensor.matmul(A0[:], clt[:, t, k * P:(k + 1) * P],
                                             w0[:, k, :], start=(k == 0),
                                             stop=(k == KT_H - 1))
                        A1 = psS.tile([P, 4 * P], f32, tag="s")
                        for k in range(KT_H):
                            nc.tensor.matmul(A1[:], cnt[:, t, k * P:(k + 1) * P],
                                             w1[:, k, :], start=(k == 0), stop=False)
                        for k in range(KT_H):
                            nc.tensor.matmul(A1[:], ctn[:, t, k * P:(k + 1) * P],
                                             w3[:, k, :], start=False,
                                             stop=(k == KT_H - 1))
                        a_tmp = wpool.tile([P, NHID], bf16, tag="atmp")
                        nc.scalar.activation(a_tmp[:], A1[:], AF.Identity,
                                             scale=r_sb[:, t:t + 1])
                        nc.vector.tensor_tensor(out=a_sb[:, t, :], in0=a_tmp[:],
                                                in1=A0[:], op=ALU.add)

                    # segment sum via one-hot matmul
                    sg = psV.tile([NG, NHID], f32, tag="pv")
                    for t in range(CT):
                        nc.tensor.matmul(sg[:], oh_seg[:, t, :], a_sb[:, t, :],
                                         start=(t == 0), stop=(t == CT - 1))
                    seg_sb = wpool.tile([NG, NHID], f32, tag="seg")
                    nc.scalar.copy(seg_sb[:], sg[:])
                    nc.sync.dma_start(seg_d[:], seg_sb[:])

                if reps == 1:
                    body()
                else:
                    unroll = 8 if reps % 8 == 0 else 1
                    with tc.For_i(0, reps // unroll):
                        for _ in range(unroll):
                            body()
    _split_excess_waits(nc)
    return nc


def make_in_maps(inputs: dict):
    """Host-side gather/transpose/convert + per-core input maps."""
    batch = np.asarray(inputs["batch"]).astype(np.int64)
    ci = np.asarray(inputs["claim_index"]).astype(np.int64)
    ei = np.asarray(inputs["evidence_index"]).astype(np.int64)
    x = np.asarray(inputs["x"], dtype=np.float32)
    cb = batch[ci]
    eb = batch[ei]
    counts = np.bincount(cb, minlength=NG).astype(np.float32)
    ba = np.asarray(inputs["ba"], dtype=np.float32).reshape(NHID)

    order = np.argsort(cb, kind="stable")
    ci, cb = ci[order], cb[order]
    eorder = np.argsort(eb, kind="stable")
    ei, eb = ei[eorder], eb[eorder]

    Wc = np.asarray(inputs["Wc"], np.float32)
    We = np.asarray(inputs["We"], np.float32)
    Wa = np.asarray(inputs["Wa"], np.float32)
    W0 = (Wa[0:NHID] + Wa[2 * NHID:3 * NHID]).astype(BF)
    W1 = (Wa[NHID:2 * NHID] - Wa[2 * NHID:3 * NHID]).astype(BF)
    W3 = Wa[3 * NHID:].astype(BF)

    def ktile(w):  # [K, N] -> [128, KT*N] with K split over partitions
        k, n = w.shape
        return np.ascontiguousarray(
            w.reshape(k // P, P, n).transpose(1, 0, 2).reshape(P, -1))

    common = {
        "w0": ktile(W0), "w1": ktile(W1), "w3": ktile(W3),
        "wcb": ktile(Wc.astype(BF)), "web": ktile(We.astype(BF)),
        "bc": np.asarray(inputs["bc"], np.float32).reshape(PROJ, 1),
        "be": np.asarray(inputs["be"], np.float32).reshape(PROJ, 1),
    }

    # per-core evidence / claims + per-tile windows
    cores = []
    max_nev = 0
    for c in range(N_CORES):
        sl = slice(c * NC_LOC, (c + 1) * NC_LOC)
        cbs = cb[sl]
        sel = np.where(np.isin(eb, np.unique(cbs)))[0]
        cores.append((ci[sl], cbs, ei[sel], eb[sel]))
        max_nev = max(max_nev, len(sel))
    ne_loc = -(-max_nev // P) * P
    ET = ne_loc // P

    # union windows (in 128-subtiles) per claim tile
    windows = []
    for t in range(CT):
        lo_s, hi_s = ET, 0
        for (_, cbs, _, ebc) in cores:
            tg = cbs[t * P:(t + 1) * P]
            lo = np.searchsorted(ebc, tg[0], "left")
            hi = np.searchsorted(ebc, tg[-1], "right")
            lo_s = min(lo_s, int(lo) // P)
            hi_s = max(hi_s, -(-int(hi) // P))
        windows.append((lo_s, max(hi_s - lo_s, 1)))
    windows = tuple(windows)

    in_maps = []
    for c in range(N_CORES):
        cid, cbs, eidx, ebc = cores[c]
        nev = len(eidx)
        ev = np.zeros((ne_loc, NHID), np.float32)
        ev[:nev] = x[eidx]
        ebp = np.full(ne_loc, -2.0, np.float32)
        ebp[:nev] = ebc
        cl = x[cid]  # [512, 512]

        evb = ev.astype(BF)
        m = dict(common)
        m["evn"] = np.ascontiguousarray(
            evb.reshape(ET, P, NHID).transpose(1, 0, 2).reshape(P, -1))
        m["evt"] = np.ascontiguousarray(
            evb.T.reshape(KT_H, P, ne_loc).transpose(1, 0, 2).reshape(P, -1))
        clb = cl.astype(BF)
        # clt[p, t, h*128+j] = cl[t*128+j, h*128+p]
        m["clt"] = np.ascontiguousarray(
            clb.reshape(CT, P, KT_H, P).transpose(3, 0, 2, 1).reshape(P, -1))
        # clk[p, k, t*128+j] = cl[t*128+j, k*128+p]
        m["clk"] = np.ascontiguousarray(
            clb.T.reshape(KT_H, P, NC_LOC).transpose(1, 0, 2).reshape(P, -1))
        m["ebr"] = ebp.astype(BF).reshape(1, ne_loc)
        m["cbr"] = cbs.astype(np.float32).astype(BF).reshape(1, NC_LOC)
        m["cbc"] = np.ascontiguousarray(
            cbs.astype(np.float32).reshape(CT, P).T)
        in_maps.append(m)
    return in_maps, counts, ba, ne_loc, windows


def postprocess(results: list, counts: np.ndarray, ba: np.ndarray) -> np.ndarray:
    seg = np.zeros((NG, NHID), np.float64)
    for c in range(N_CORES):
        seg += results[c]["seg"].astype(np.float64)
    # segment_mean(a + ba) = segment_mean(a) + ba, except empty graphs stay 0
    out = seg / np.maximum(counts, 1.0)[:, None] + (counts > 0)[:, None] * ba[None, :]
    return out.astype(np.float32)


def kernel(**inputs) -> np.ndarray:
    in_maps, counts, ba, ne_loc, windows = make_in_maps(inputs)
    nc = build_nc(reps=1, ne_loc=ne_loc, windows=windows)
    res = run_bass_kernel_spmd(nc, in_maps, list(range(N_CORES)))
    return postprocess(res.results, counts, ba)
